# revision 25
# baseline (speedup 1.0000x reference)
"""Trainium2 Bass kernel for nn_ExampleTiedDropout (scatter_memory).

reference: out = X * mask[:, :, None] where mask[b] has the first
int(0.2*S)=204 positions fixed to 1 and the remaining 820 positions
Bernoulli(0.1) keyed by fold_in(key(0), idx[b]).

Since the mask is exactly {0, 1}, the output is a row-sparse copy of X:
~28% of the (b, s) rows are copied verbatim, the rest are zero. The
kernel:
  1. computes the mask on host with the same jax ops as the reference
     (bit-exact: same env -> same rbg backend bitstream),
  2. shards the batch data-parallel across 8 NeuronCores (4 examples =
     4096 rows of 2048 floats per core),
  3. copies the always-kept fixed prefix (204 rows/example, contiguous,
     71% of kept bytes) as bulk partition-mapped DMAs: examples 0-2 on
     the two HWDGE rings (loads on SP, stores on ACT), example 3 on the
     SWDGE queue -- three DMA queues running concurrently,
  4. moves the ~330 scattered kept rows per core with indirect
     gather/scatter DMAs driven by a host-built row-index table (padded
     entries are out-of-bounds and skipped),
  5. leaves dropped rows untouched -- ExternalOutput buffers are donated
     zero-filled buffers, so unwritten rows read back as 0.
Raw Bass engine blocks with manual semaphores (no TileContext): Tile
inserts a false WAW dependency that serializes the scatters behind all
fixed stores; manual sems let all three queues run concurrently.
"""
import numpy as np

B, S, H = 32, 1024, 2048
N_CORES = 8
BPC = B // N_CORES           # examples per core
ROWS = BPC * S               # rows of H floats per core
P = 128                      # SBUF partitions
P_FIXED, P_MEM, MASK_SEED = 0.2, 0.1, 0
N_FIXED = int(P_FIXED * S)   # 204 leading rows per example, always kept
FW = N_FIXED * H // P        # fixed block viewed as [128, FW] (3264)
OOB_IDX = 1 << 24            # padded index; > bounds_check -> DMA skips it
QW = FW // 4                 # fixed blocks move in quarter chunks (816 cols)

_PROGRAM_CACHE = {}
LAST_RESULTS = None


def _ensure_ntff_hook():
    """The concourse trace path imports antenv.axon_hooks, which this image's
    antenv package lacks -- a hard crash when tracing is requested. Provide
    the missing module and register the boot's ctypes-based hook so NTFF
    profiling works as designed. No-op when the real module exists."""
    try:
        import antenv.axon_hooks  # noqa: F401
        return
    except ImportError:
        pass
    import sys
    import types

    mod = types.ModuleType("antenv.axon_hooks")
    mod._hook = None
    mod.set_axon_ntff_profile_hook = lambda h: setattr(mod, "_hook", h)
    mod.get_axon_ntff_profile_hook = lambda: mod._hook
    sys.modules["antenv.axon_hooks"] = mod
    try:
        import antenv
        antenv.axon_hooks = mod
    except ImportError:
        pass
    try:
        from trn_agent_boot.trn_boot import _ntff_profile_via_ctypes
        mod._hook = _ntff_profile_via_ctypes("/opt/axon/libaxon_pjrt.so")
    except Exception:
        pass  # hook stays None: concourse logs a warning and skips tracing


_ensure_ntff_hook()


def _tied_dropout_mask_host(idx_np):
    """Verbatim replica of reference._tied_dropout_mask, evaluated with the
    process-default jax backend/PRNG so the bits match the grader's
    reference run in the same environment."""
    import jax
    import jax.numpy as jnp

    n_fixed = int(P_FIXED * S)
    n_rand = S - n_fixed
    base = jax.random.key(MASK_SEED)

    def row_mask(i):
        k = jax.random.fold_in(base, i)
        return jax.random.bernoulli(k, P_MEM, (n_rand,)).astype(jnp.float32)

    idx = jnp.asarray(idx_np)
    rand_part = jax.vmap(row_mask)(idx)
    fixed_part = jnp.ones((idx.shape[0], n_fixed), jnp.float32)
    return np.asarray(jnp.concatenate([fixed_part, rand_part], axis=1))


def _fixed_view(ap, e, q):
    """Quarter q of example e's fixed prefix as a [128, FW/4] partition-
    mapped view of the contiguous block (rows e*S .. e*S+N_FIXED)."""
    blk = ap[e * S:e * S + N_FIXED, :]
    flat = blk.rearrange("s h -> (s h)").rearrange("(p f) -> p f", p=P)
    return flat[:, q * QW:(q + 1) * QW]


def _build_program(n_tiles):
    """Raw-Bass SPMD program per core. Three concurrent DMA streams:
      SP ring   : fixed loads, examples 0..2 (x -> SBUF, 2 halves each)
      ACT ring  : fixed stores, examples 0..2 (SBUF -> y, after own load)
      SWDGE q0  : idx load, example 3 fixed copy, n_tiles x (indirect
                  gather 128 rows -> SBUF, indirect scatter -> y)
    Value-agnostic: row choices live in the kidx tensor; padded entries
    are out-of-bounds and skipped by the DMA engine."""
    from contextlib import ExitStack

    import concourse.bacc as bacc
    import concourse.bass as bass
    from concourse import mybir

    nt = n_tiles
    nc = bacc.Bacc("TRN2", target_bir_lowering=False, debug=False,
                   num_devices=N_CORES)
    x = nc.dram_tensor("x", [ROWS, H], mybir.dt.float32, kind="ExternalInput")
    kidx = nc.dram_tensor("kidx", [P, max(nt, 1)], mybir.dt.int32,
                          kind="ExternalInput")
    y = nc.dram_tensor("y", [ROWS, H], mybir.dt.float32, kind="ExternalOutput")

    # Each wait below is exact: a semaphore's threshold 16*k is reached only
    # when all k DMAs that increment it have fully completed (a shared
    # counter across more DMAs could hit the threshold with partial
    # completions from later transfers).
    # Work split: fixed blocks move as direct DRAM->DRAM quarter copies (no
    # SBUF bounce, no load->store chain), balanced so the three queues
    # finish together:
    #   SP ring  : e0 all quarters + e1 quarters 0-2
    #   ACT ring : e2 all quarters + e3 quarters 0-2
    #   SWDGE q0 : idx load, e1q3 + e3q3 copies, indirect gathers+scatters
    with ExitStack() as ctx:
        xts = ctx.enter_context(
            nc.sbuf_tensor([P, max(nt, 1) * H], mybir.dt.float32))
        it = ctx.enter_context(
            nc.sbuf_tensor([P, max(nt, 1)], mybir.dt.int32))
        s_cp1 = ctx.enter_context(nc.semaphore("s_cp1"))  # SP D2D copies
        s_cp2 = ctx.enter_context(nc.semaphore("s_cp2"))  # ACT D2D copies
        s_cpq = ctx.enter_context(nc.semaphore("s_cpq"))  # q0 D2D copies
        s_idx = ctx.enter_context(nc.semaphore("s_idx"))
        s_g = [ctx.enter_context(nc.semaphore(f"s_g{t}"))
               for t in range(nt)]                  # one per gather
        s_out = ctx.enter_context(nc.semaphore("s_out"))  # q0 scatters
        block = ctx.enter_context(nc.Block(no_gpsimd_drain=True))

        @block.sync
        def _(sync):
            if nt > 0:
                sync.dma_start(out=it[:], in_=kidx[:]).then_inc(s_idx, 16)
            for e in (0, 1):
                for q in range(4):
                    sync.dma_start(
                        out=_fixed_view(y, e, q), in_=_fixed_view(x, e, q)
                    ).then_inc(s_cp1, 16)
            sync.wait_ge(s_cp1, 16 * 8)

        @block.scalar
        def _(scalar):
            for e in (2, 3):
                for q in range(4):
                    scalar.dma_start(
                        out=_fixed_view(y, e, q), in_=_fixed_view(x, e, q)
                    ).then_inc(s_cp2, 16)
            scalar.wait_ge(s_cp2, 16 * 8)

        @block.gpsimd
        def _(gpsimd):
            # idx table loads on the SP ring in parallel; q0 is purely
            # the indirect gather/scatter pipeline.
            for t in range(nt):
                if t == 0:
                    gpsimd.wait_ge(s_idx, 16)
                gpsimd.indirect_dma_start(
                    out=xts[:, t * H:(t + 1) * H],
                    out_offset=None,
                    in_=x[:],
                    in_offset=bass.IndirectOffsetOnAxis(
                        ap=it[:, t:t + 1], axis=0),
                    bounds_check=ROWS - 1,
                    oob_is_err=False,
                ).then_inc(s_g[t], 16)
            for t in range(nt):
                gpsimd.wait_ge(s_g[t], 16)
                gpsimd.indirect_dma_start(
                    out=y[:],
                    out_offset=bass.IndirectOffsetOnAxis(
                        ap=it[:, t:t + 1], axis=0),
                    in_=xts[:, t * H:(t + 1) * H],
                    in_offset=None,
                    bounds_check=ROWS - 1,
                    oob_is_err=False,
                ).then_inc(s_out, 16)
            if nt > 0:
                gpsimd.wait_ge(s_out, 16 * nt)

    nc.compile()
    return nc


def kernel(X, idx):
    global LAST_RESULTS
    from concourse.bass_utils import run_bass_kernel_spmd

    X = np.ascontiguousarray(np.asarray(X, dtype=np.float32))
    idx = np.asarray(idx, dtype=np.int32)

    mask = _tied_dropout_mask_host(idx)          # [B, S] float32 of {0,1}
    keep = mask.reshape(N_CORES, ROWS) > 0.5     # [8, 4096] bool
    # The fixed prefix rows (s < N_FIXED of each example) are copied by the
    # static bulk DMAs; only scattered kept rows go through the index table.
    keep[:, :] &= np.tile(np.arange(S) >= N_FIXED, BPC)[None, :]

    keep_rows = [np.flatnonzero(keep[c]).astype(np.int32) for c in range(N_CORES)]
    max_keep = max(len(r) for r in keep_rows)
    n_tiles = -(-max_keep // P)                  # same static tile count per core

    in_maps = []
    for c in range(N_CORES):
        nt = max(n_tiles, 1)
        r = keep_rows[c]
        # scatter/gather table: int32, OOB padding (skipped); tile t = col t
        flat = np.full((nt * P,), OOB_IDX, dtype=np.int32)
        flat[: len(r)] = r
        kidx = np.ascontiguousarray(flat.reshape(nt, P).T)
        in_maps.append({
            "x": X[c * BPC:(c + 1) * BPC].reshape(ROWS, H),
            "kidx": kidx,
        })

    if n_tiles not in _PROGRAM_CACHE:
        _PROGRAM_CACHE[n_tiles] = _build_program(n_tiles)
    nc = _PROGRAM_CACHE[n_tiles]

    res = run_bass_kernel_spmd(nc, in_maps, list(range(N_CORES)))
    LAST_RESULTS = res

    out = np.empty((B, S, H), dtype=np.float32)
    for c in range(N_CORES):
        out[c * BPC:(c + 1) * BPC] = res.results[c]["y"].reshape(BPC, S, H)
    return out


# revision 26
# speedup vs baseline: 1.0738x; 1.0738x over previous
"""Trainium2 Bass kernel for nn_ExampleTiedDropout (scatter_memory).

reference: out = X * mask[:, :, None] where mask[b] has the first
int(0.2*S)=204 positions fixed to 1 and the remaining 820 positions
Bernoulli(0.1) keyed by fold_in(key(0), idx[b]).

Since the mask is exactly {0, 1}, the output is a row-sparse copy of X:
~28% of the (b, s) rows are copied verbatim, the rest are zero. The
kernel:
  1. computes the mask on host with the same jax ops as the reference
     (bit-exact: same env -> same rbg backend bitstream),
  2. shards the batch data-parallel across 8 NeuronCores (4 examples =
     4096 rows of 2048 floats per core),
  3. copies the always-kept fixed prefix (204 rows/example, contiguous,
     71% of kept bytes) as bulk partition-mapped DMAs: examples 0-2 on
     the two HWDGE rings (loads on SP, stores on ACT), example 3 on the
     SWDGE queue -- three DMA queues running concurrently,
  4. moves the ~330 scattered kept rows per core with indirect
     gather/scatter DMAs driven by a host-built row-index table (padded
     entries are out-of-bounds and skipped),
  5. leaves dropped rows untouched -- ExternalOutput buffers are donated
     zero-filled buffers, so unwritten rows read back as 0.
Raw Bass engine blocks with manual semaphores (no TileContext): Tile
inserts a false WAW dependency that serializes the scatters behind all
fixed stores; manual sems let all three queues run concurrently.
"""
import numpy as np

B, S, H = 32, 1024, 2048
N_CORES = 8
BPC = B // N_CORES           # examples per core
ROWS = BPC * S               # rows of H floats per core
P = 128                      # SBUF partitions
P_FIXED, P_MEM, MASK_SEED = 0.2, 0.1, 0
N_FIXED = int(P_FIXED * S)   # 204 leading rows per example, always kept
FW = N_FIXED * H // P        # fixed block viewed as [128, FW] (3264)
OOB_IDX = 1 << 24            # padded index; > bounds_check -> DMA skips it
QW = FW // 4                 # fixed blocks move in quarter chunks (816 cols)

_PROGRAM_CACHE = {}
LAST_RESULTS = None


def _ensure_ntff_hook():
    """The concourse trace path imports antenv.axon_hooks, which this image's
    antenv package lacks -- a hard crash when tracing is requested. Provide
    the missing module and register the boot's ctypes-based hook so NTFF
    profiling works as designed. No-op when the real module exists."""
    try:
        import antenv.axon_hooks  # noqa: F401
        return
    except ImportError:
        pass
    import sys
    import types

    mod = types.ModuleType("antenv.axon_hooks")
    mod._hook = None
    mod.set_axon_ntff_profile_hook = lambda h: setattr(mod, "_hook", h)
    mod.get_axon_ntff_profile_hook = lambda: mod._hook
    sys.modules["antenv.axon_hooks"] = mod
    try:
        import antenv
        antenv.axon_hooks = mod
    except ImportError:
        pass
    try:
        from trn_agent_boot.trn_boot import _ntff_profile_via_ctypes
        mod._hook = _ntff_profile_via_ctypes("/opt/axon/libaxon_pjrt.so")
    except Exception:
        pass  # hook stays None: concourse logs a warning and skips tracing


_ensure_ntff_hook()


def _tied_dropout_mask_host(idx_np):
    """Verbatim replica of reference._tied_dropout_mask, evaluated with the
    process-default jax backend/PRNG so the bits match the grader's
    reference run in the same environment."""
    import jax
    import jax.numpy as jnp

    n_fixed = int(P_FIXED * S)
    n_rand = S - n_fixed
    base = jax.random.key(MASK_SEED)

    def row_mask(i):
        k = jax.random.fold_in(base, i)
        return jax.random.bernoulli(k, P_MEM, (n_rand,)).astype(jnp.float32)

    idx = jnp.asarray(idx_np)
    rand_part = jax.vmap(row_mask)(idx)
    fixed_part = jnp.ones((idx.shape[0], n_fixed), jnp.float32)
    return np.asarray(jnp.concatenate([fixed_part, rand_part], axis=1))


def _fixed_view(ap, e, q):
    """Quarter q of example e's fixed prefix as a [128, FW/4] partition-
    mapped view of the contiguous block (rows e*S .. e*S+N_FIXED)."""
    blk = ap[e * S:e * S + N_FIXED, :]
    flat = blk.rearrange("s h -> (s h)").rearrange("(p f) -> p f", p=P)
    return flat[:, q * QW:(q + 1) * QW]


def _build_program(n_tiles):
    """Raw-Bass SPMD program per core. Three concurrent DMA streams:
      SP ring   : fixed loads, examples 0..2 (x -> SBUF, 2 halves each)
      ACT ring  : fixed stores, examples 0..2 (SBUF -> y, after own load)
      SWDGE q0  : idx load, example 3 fixed copy, n_tiles x (indirect
                  gather 128 rows -> SBUF, indirect scatter -> y)
    Value-agnostic: row choices live in the kidx tensor; padded entries
    are out-of-bounds and skipped by the DMA engine."""
    from contextlib import ExitStack

    import concourse.bacc as bacc
    import concourse.bass as bass
    from concourse import mybir

    nt = n_tiles
    nc = bacc.Bacc("TRN2", target_bir_lowering=False, debug=False,
                   num_devices=N_CORES)
    x = nc.dram_tensor("x", [ROWS, H], mybir.dt.float32, kind="ExternalInput")
    kidx = nc.dram_tensor("kidx", [P, max(nt, 1)], mybir.dt.int32,
                          kind="ExternalInput")
    y = nc.dram_tensor("y", [ROWS, H], mybir.dt.float32, kind="ExternalOutput")

    # Each wait below is exact: a semaphore's threshold 16*k is reached only
    # when all k DMAs that increment it have fully completed (a shared
    # counter across more DMAs could hit the threshold with partial
    # completions from later transfers).
    # Work split: fixed blocks move as direct DRAM->DRAM quarter copies (no
    # SBUF bounce, no load->store chain), balanced so the three queues
    # finish together:
    #   SP ring  : e0 all quarters + e1 quarters 0-2
    #   ACT ring : e2 all quarters + e3 quarters 0-2
    #   SWDGE q0 : idx load, e1q3 + e3q3 copies, indirect gathers+scatters
    with ExitStack() as ctx:
        xts = ctx.enter_context(
            nc.sbuf_tensor([P, max(nt, 1) * H], mybir.dt.float32))
        it = ctx.enter_context(
            nc.sbuf_tensor([P, max(nt, 1)], mybir.dt.int32))
        s_cp1 = ctx.enter_context(nc.semaphore("s_cp1"))  # SP D2D copies
        s_cp2 = ctx.enter_context(nc.semaphore("s_cp2"))  # ACT D2D copies
        s_cpq = ctx.enter_context(nc.semaphore("s_cpq"))  # q0 D2D copies
        s_idx = ctx.enter_context(nc.semaphore("s_idx"))
        s_g = [ctx.enter_context(nc.semaphore(f"s_g{t}"))
               for t in range(nt)]                  # one per gather
        s_out = ctx.enter_context(nc.semaphore("s_out"))  # q0 scatters
        block = ctx.enter_context(nc.Block(no_gpsimd_drain=True))

        @block.sync
        def _(sync):
            for e in (0, 1):
                for q in range(4):
                    sync.dma_start(
                        out=_fixed_view(y, e, q), in_=_fixed_view(x, e, q)
                    ).then_inc(s_cp1, 16)
            sync.wait_ge(s_cp1, 16 * 8)

        @block.scalar
        def _(scalar):
            for e in (2, 3):
                for q in range(4):
                    scalar.dma_start(
                        out=_fixed_view(y, e, q), in_=_fixed_view(x, e, q)
                    ).then_inc(s_cp2, 16)
            scalar.wait_ge(s_cp2, 16 * 8)

        @block.gpsimd
        def _(gpsimd):
            # idx first: its ~5us completion latency overlaps the D2D
            # copies draining through the queue.
            if nt > 0:
                gpsimd.dma_start(out=it[:], in_=kidx[:]).then_inc(s_idx, 16)
            for t in range(nt):
                if t == 0:
                    gpsimd.wait_ge(s_idx, 16)
                gpsimd.indirect_dma_start(
                    out=xts[:, t * H:(t + 1) * H],
                    out_offset=None,
                    in_=x[:],
                    in_offset=bass.IndirectOffsetOnAxis(
                        ap=it[:, t:t + 1], axis=0),
                    bounds_check=ROWS - 1,
                    oob_is_err=False,
                ).then_inc(s_g[t], 16)
            for t in range(nt):
                gpsimd.wait_ge(s_g[t], 16)
                gpsimd.indirect_dma_start(
                    out=y[:],
                    out_offset=bass.IndirectOffsetOnAxis(
                        ap=it[:, t:t + 1], axis=0),
                    in_=xts[:, t * H:(t + 1) * H],
                    in_offset=None,
                    bounds_check=ROWS - 1,
                    oob_is_err=False,
                ).then_inc(s_out, 16)
            if nt > 0:
                gpsimd.wait_ge(s_out, 16 * nt)

    nc.compile()
    return nc


def kernel(X, idx):
    global LAST_RESULTS
    from concourse.bass_utils import run_bass_kernel_spmd

    X = np.ascontiguousarray(np.asarray(X, dtype=np.float32))
    idx = np.asarray(idx, dtype=np.int32)

    mask = _tied_dropout_mask_host(idx)          # [B, S] float32 of {0,1}
    keep = mask.reshape(N_CORES, ROWS) > 0.5     # [8, 4096] bool
    # The fixed prefix rows (s < N_FIXED of each example) are copied by the
    # static bulk DMAs; only scattered kept rows go through the index table.
    keep[:, :] &= np.tile(np.arange(S) >= N_FIXED, BPC)[None, :]

    keep_rows = [np.flatnonzero(keep[c]).astype(np.int32) for c in range(N_CORES)]
    max_keep = max(len(r) for r in keep_rows)
    n_tiles = -(-max_keep // P)                  # same static tile count per core

    in_maps = []
    for c in range(N_CORES):
        nt = max(n_tiles, 1)
        r = keep_rows[c]
        # scatter/gather table: int32, OOB padding (skipped); tile t = col t
        flat = np.full((nt * P,), OOB_IDX, dtype=np.int32)
        flat[: len(r)] = r
        kidx = np.ascontiguousarray(flat.reshape(nt, P).T)
        in_maps.append({
            "x": X[c * BPC:(c + 1) * BPC].reshape(ROWS, H),
            "kidx": kidx,
        })

    if n_tiles not in _PROGRAM_CACHE:
        _PROGRAM_CACHE[n_tiles] = _build_program(n_tiles)
    nc = _PROGRAM_CACHE[n_tiles]

    res = run_bass_kernel_spmd(nc, in_maps, list(range(N_CORES)))
    LAST_RESULTS = res

    out = np.empty((B, S, H), dtype=np.float32)
    for c in range(N_CORES):
        out[c * BPC:(c + 1) * BPC] = res.results[c]["y"].reshape(BPC, S, H)
    return out
